# revision 12
# baseline (speedup 1.0000x reference)
"""BiLSTM translator kernel for 8 trn2 NeuronCores.

Sharding/layout strategy:
  - Activations feature-major ("T"): [feature -> 128-row chunks, tokens].
    Encoder tokens ordered (b, s) -> col b*64+s; decoder tokens (t, b) -> col t*32+b.
  - Weights bf16 stationary lhsT tiles [128,128]; fp32 PSUM accumulation; fp32 cell state.
  - Encoder: replicated on all 8 cores; input transforms (gx) batched over tokens into DRAM.
  - Decoder: gate-columns sharded 8-way (core c owns rows 128c..128c+128 of each gate);
    per-step AllGather stacks slices on the partition axis == feature-major concat.
  - Output projection: vocab sharded 8-way (4000 rows/core, zero-padded to 4096).
"""

import numpy as np
import ml_dtypes

import concourse.bass as bass
import concourse.mybir as mybir
import concourse.tile as tile

B, S, T = 32, 64, 48
VS, VT, E, H = 32000, 32000, 256, 512
H2 = 2 * H
NC = 8
NS = B * S
NT = B * T
VP = 4096
BF = mybir.dt.bfloat16
F32 = mybir.dt.float32
I32 = mybir.dt.int32

SIG = mybir.ActivationFunctionType.Sigmoid
TANH = mybir.ActivationFunctionType.Tanh
EXP = mybir.ActivationFunctionType.Exp
ADD = mybir.AluOpType.add
MUL = mybir.AluOpType.mult


# ----------------------------------------------------------------------------
# host-side prep
# ----------------------------------------------------------------------------

def _bf(x):
    return np.asarray(np.asarray(x, np.float32).astype(ml_dtypes.bfloat16))


def _kstack(w):
    """[K, N] -> [128, K//128, N] stationary-tile layout."""
    k, n = w.shape
    assert k % 128 == 0
    return np.ascontiguousarray(w.reshape(k // 128, 128, n).transpose(1, 0, 2))


def _fmbias(v):
    """bias [D] -> [128, D//128, 1] fp32."""
    d = v.shape[0]
    return np.ascontiguousarray(
        np.asarray(v, np.float32).reshape(d // 128, 128, 1).transpose(1, 0, 2))


def prepare_in_maps(src, tgt, emb_src, emb_tgt, enc_params, dec_params, ac_w, ac_b, op_w, op_b):
    src = np.asarray(src).astype(np.int32)
    tgt = np.asarray(tgt).astype(np.int32)
    common = {}
    common["src_idx"] = np.ascontiguousarray(src.reshape(NS, 1))            # (b,s)
    common["tgt_idx"] = np.ascontiguousarray(tgt.T.reshape(NT, 1))          # (t,b)
    common["emb_src"] = _bf(emb_src)
    common["emb_tgt"] = _bf(emb_tgt)

    for li, lp in enumerate(enc_params):
        for di, off in ((0, 0), (1, 4)):
            wih, whh, bih, bhh = (np.asarray(a) for a in lp[off:off + 4])
            tag = f"e{li}{'fb'[di]}"
            common[f"wx_{tag}"] = _bf(_kstack(wih.T))       # [128, din/128, 2048]
            common[f"wh_{tag}"] = _bf(_kstack(whh.T))       # [128, 4, 2048]
            common[f"b_{tag}"] = _fmbias(bih + bhh)         # [128, 16, 1]

    (w0, u0, b0, d0), (w1, u1, b1, d1) = dec_params
    w0, u0, w1, u1 = (np.asarray(a) for a in (w0, u0, w1, u1))
    bd0 = np.asarray(b0) + np.asarray(d0)
    bd1 = np.asarray(b1) + np.asarray(d1)

    per_core = [dict() for _ in range(NC)]
    for c in range(NC):
        cols = np.concatenate(
            [np.arange(g * H2 + 128 * c, g * H2 + 128 * (c + 1)) for g in range(4)])
        per_core[c]["wx_d0"] = _bf(_kstack(w0.T[:, cols]))      # [128, 2, 512]
        per_core[c]["wh_d0"] = _bf(_kstack(u0.T[:, cols]))      # [128, 8, 512]
        per_core[c]["b_d0"] = _fmbias(bd0[cols])                # [128, 4, 1]
        wc1 = np.concatenate([w1.T[:, cols], u1.T[:, cols]], axis=0)
        per_core[c]["w_d1"] = _bf(_kstack(wc1))                 # [128, 16, 512]
        per_core[c]["b_d1"] = np.ascontiguousarray(
            np.repeat(_fmbias(bd1[cols]), B, axis=2))           # [128, 4, 32]
        m = np.zeros((8, 128, B), np.float32)
        m[c] = 1.0
        per_core[c]["selmask"] = np.ascontiguousarray(m.transpose(1, 0, 2))  # [128, 8, 32]
        vlo = c * 4000
        opw = np.zeros((H2, VP), np.float32)
        opw[:, :4000] = np.asarray(op_w).T[:, vlo:vlo + 4000]
        per_core[c]["opw"] = _bf(_kstack(opw))                  # [128, 8, 4096]
        opb = np.zeros((VP,), np.float32)
        opb[:4000] = np.asarray(op_b)[vlo:vlo + 4000]
        per_core[c]["opb"] = _fmbias(opb)                       # [128, 32, 1]

    common["acw"] = _bf(_kstack(np.asarray(ac_w).T))            # [128, 16, 1024]
    common["acb"] = _fmbias(np.asarray(ac_b))                   # [128, 8, 1]
    return [dict(common, **per_core[c]) for c in range(NC)]


def split_drain_waits(nc):
    """This walrus accepts at most one sync wait per instruction: hoist extras onto NoOps."""
    nid = [0]
    for fn in nc.m.functions:
        for blk in fn.blocks:
            newinsts = []
            for ins in blk.instructions:
                si = ins.sync_info
                if si and si.on_wait and len(si.on_wait) > 1:
                    for w in si.on_wait[:-1]:
                        nid[0] += 1
                        nop = mybir.InstNoOp(
                            name=f"I-dsplit-{nid[0]}", ins=[], outs=[],
                            sync_info=mybir.SyncInfo(on_wait=[w], on_update=[]))
                        nop.engine = ins.engine
                        newinsts.append(nop)
                    si.on_wait = si.on_wait[-1:]
                newinsts.append(ins)
            blk.instructions[:] = newinsts


# ----------------------------------------------------------------------------
# device program pieces
# ----------------------------------------------------------------------------

def _lstm_chain(nc, pool, g_sb, c_st, h_out_ap, nch):
    """Cell update from gates g_sb [128, 4*nch, B] (i,f,g,o groups), fp32 c_st [128, nch, B]."""
    sif = pool.tile([128, 2 * nch, B], BF, tag="sif", name="sif")
    tg = pool.tile([128, nch, B], BF, tag="tg", name="tg")
    so = pool.tile([128, nch, B], BF, tag="so", name="so")
    mm = pool.tile([128, nch, B], BF, tag="mm", name="mm")
    tcc = pool.tile([128, nch, B], BF, tag="tcc", name="tcc")
    nc.scalar.activation(sif[:], g_sb[:, 0:2 * nch, :], SIG)
    nc.scalar.activation(tg[:], g_sb[:, 2 * nch:3 * nch, :], TANH)
    nc.scalar.activation(so[:], g_sb[:, 3 * nch:4 * nch, :], SIG)
    nc.vector.tensor_tensor(out=mm[:], in0=sif[:, 0:nch, :], in1=tg[:], op=MUL)
    nc.vector.tensor_tensor(out=c_st[:], in0=sif[:, nch:2 * nch, :], in1=c_st[:], op=MUL)
    nc.vector.tensor_tensor(out=c_st[:], in0=c_st[:], in1=mm[:], op=ADD)
    nc.scalar.activation(tcc[:], c_st[:], TANH)
    nc.vector.tensor_tensor(out=h_out_ap, in0=so[:], in1=tcc[:], op=MUL)


def _enc_pass(nc, tc, wp, wx, wh, bb, gx_dram, cfin, li, d, src_tile, dst_tile, nkx):
    """One encoder direction pass: batched gx -> DRAM, then the recurrence."""
    tag = f"e{li}{d}"
    with tc.tile_pool(name=f"enc_{tag}", bufs=1) as encp, \
         tc.tile_pool(name=f"ps_{tag}", bufs=2, space="PSUM") as pse2:
        wxs = encp.tile([128, nkx, 4 * H], BF, name="wxs")
        whs = encp.tile([128, 4, 4 * H], BF, name="whs")
        bs = encp.tile([128, 16, 1], F32, name="bs")
        nc.sync.dma_start(wxs[:], wx[tag][:])
        nc.sync.dma_start(whs[:], wh[tag][:])
        nc.sync.dma_start(bs[:], bb[tag][:])

        # batched gx -> DRAM, token tiles = 16 s-values x 32 b
        srcv = src_tile[:].rearrange("p k (b s) -> p k s b", s=S)
        for ts in range(4):
            for m in range(16):
                ps = pse2.tile([128, 512], F32, tag="gx_ps", name="gx_ps")
                for k in range(nkx):
                    nc.tensor.matmul(
                        ps[:], wxs[:, k, m * 128:(m + 1) * 128],
                        srcv[:, k, ts * 16:(ts + 1) * 16, :],
                        start=(k == 0), stop=(k == nkx - 1))
                tmp = wp.tile([128, 512], BF, tag="gxev", name="gxev")
                nc.vector.tensor_tensor(
                    out=tmp[:], in0=ps[:],
                    in1=bs[:, m, 0:1].to_broadcast([128, 512]), op=ADD)
                nc.sync.dma_start(
                    gx_dram[:, m, ts * 16:(ts + 1) * 16, :].rearrange("p s b -> p (s b)"),
                    tmp[:])

        hT = wp.tile([128, 4, B], BF, tag="hT", name="hT", bufs=2)
        cT = cfin[(li, d)]
        nc.vector.memset(hT[:], 0.0)
        nc.vector.memset(cT[:], 0.0)
        doff = 0 if d == "f" else 4
        for step in range(S):
            s = step if d == "f" else S - 1 - step
            gxs = wp.tile([128, 16, B], BF, tag="gxs", name="gxs")
            nc.sync.dma_start(gxs[:], gx_dram[:, :, s, :])
            ps = pse2.tile([128, 512], F32, tag="enc_ps", name="enc_ps")
            for m in range(16):
                for k in range(4):
                    nc.tensor.matmul(
                        ps[:, m * 32:(m + 1) * 32],
                        whs[:, k, m * 128:(m + 1) * 128], hT[:, k, :],
                        start=(k == 0), stop=(k == 3))
            g_sb = wp.tile([128, 16, B], BF, tag="g_sb", name="g_sb")
            nc.vector.tensor_tensor(
                out=g_sb[:], in0=ps[:].rearrange("p (m b) -> p m b", b=B),
                in1=gxs[:], op=ADD)
            hT_new = wp.tile([128, 4, B], BF, tag="hT", name="hT", bufs=2)
            _lstm_chain(nc, wp, g_sb, cT, hT_new[:], 4)
            nc.vector.tensor_copy(
                out=dst_tile[:, doff:doff + 4, s::S], in_=hT_new[:])
            hT = hT_new


def build_nc(debug_taps=()):
    nc = bass.Bass(num_devices=NC)

    src_idx = nc.dram_tensor("src_idx", [NS, 1], I32, kind="ExternalInput")
    tgt_idx = nc.dram_tensor("tgt_idx", [NT, 1], I32, kind="ExternalInput")
    emb_src = nc.dram_tensor("emb_src", [VS, E], BF, kind="ExternalInput")
    emb_tgt = nc.dram_tensor("emb_tgt", [VT, E], BF, kind="ExternalInput")
    wx = {}; wh = {}; bb = {}
    for li in range(2):
        for d in "fb":
            tag = f"e{li}{d}"
            din = E if li == 0 else H2
            wx[tag] = nc.dram_tensor(f"wx_{tag}", [128, din // 128, 4 * H], BF, kind="ExternalInput")
            wh[tag] = nc.dram_tensor(f"wh_{tag}", [128, 4, 4 * H], BF, kind="ExternalInput")
            bb[tag] = nc.dram_tensor(f"b_{tag}", [128, 16, 1], F32, kind="ExternalInput")
    wx_d0 = nc.dram_tensor("wx_d0", [128, 2, 512], BF, kind="ExternalInput")
    wh_d0 = nc.dram_tensor("wh_d0", [128, 8, 512], BF, kind="ExternalInput")
    b_d0 = nc.dram_tensor("b_d0", [128, 4, 1], F32, kind="ExternalInput")
    w_d1 = nc.dram_tensor("w_d1", [128, 16, 512], BF, kind="ExternalInput")
    b_d1 = nc.dram_tensor("b_d1", [128, 4, B], F32, kind="ExternalInput")
    selmask = nc.dram_tensor("selmask", [128, 8, B], F32, kind="ExternalInput")
    acw_d = nc.dram_tensor("acw", [128, 16, H2], BF, kind="ExternalInput")
    acb_d = nc.dram_tensor("acb", [128, 8, 1], F32, kind="ExternalInput")
    opw_d = nc.dram_tensor("opw", [128, 8, VP], BF, kind="ExternalInput")
    opb_d = nc.dram_tensor("opb", [128, 32, 1], F32, kind="ExternalInput")
    logits_out = nc.dram_tensor("logits_T", [VP, NT], F32, kind="ExternalOutput")

    taps = {}
    for name, shape in debug_taps:
        taps[name] = nc.dram_tensor(name, shape, BF, kind="ExternalOutput")

    def dump_tap(wp, name, src_ap, nch, ncols):
        if name in taps:
            for ch in range(nch):
                nc.sync.dma_start(taps[name][ch * 128:(ch + 1) * 128, :], src_ap(ch))

    rg = [list(range(NC))]

    with tile.TileContext(nc) as tc:
        with tc.tile_pool(name="pp", bufs=1) as pp, \
             tc.tile_pool(name="wp", bufs=3) as wp, \
             tc.tile_pool(name="dp", bufs=2, space="DRAM") as dp:

            ident = pp.tile([128, 128], BF, name="ident")
            from concourse.masks import make_identity
            make_identity(nc, ident[:])

            # persistent-ish mid-size tiles
            gx0 = pp.tile([128, 4, T, B], BF, name="gx0")
            h1_hist = pp.tile([128, 8, T, B], BF, name="h1_hist")
            ctx_T = pp.tile([128, 8, T, B], BF, name="ctx_T")
            cfin = {(li, d): pp.tile([128, 4, B], F32, name=f"cfin_{li}{d}")
                    for li in range(2) for d in "fb"}

            gx_dram = dp.tile([128, 16, S, B], BF, name="gx_dram", bufs=1)

            with tc.tile_pool(name="enc_act", bufs=1) as ea:
                x2_T = ea.tile([128, 8, NS], BF, name="x2_T")
                enc_T = ea.tile([128, 8, NS], BF, name="enc_T")

                # ---------- phase 0: gathers ----------
                with tc.tile_pool(name="embp", bufs=1) as ep, \
                     tc.tile_pool(name="ps_emb", bufs=2, space="PSUM") as pse:
                    x_T = ep.tile([128, 2, NS], BF, name="x_T")
                    xt_T = ep.tile([128, 2, NT], BF, name="xt_T")

                    def gather_embed(idx_dram, n_tok, table, out_T):
                        idx_sb = wp.tile([128, n_tok // 128], I32, tag="idx", name="idx")
                        nc.sync.dma_start(idx_sb[:], idx_dram.rearrange("(c p) one -> p (c one)", p=128))
                        for j in range(n_tok // 128):
                            rows = wp.tile([128, E], BF, tag="erows", name="erows")
                            nc.gpsimd.indirect_dma_start(
                                out=rows[:], out_offset=None, in_=table[:],
                                in_offset=bass.IndirectOffsetOnAxis(ap=idx_sb[:, j:j + 1], axis=0))
                            for e in range(2):
                                pst = pse.tile([128, 128], BF, tag="tr_ps", name="tr_ps")
                                nc.tensor.transpose(out=pst[:], in_=rows[:, e * 128:(e + 1) * 128],
                                                    identity=ident[:])
                                nc.vector.tensor_copy(out=out_T[:, e, j * 128:(j + 1) * 128], in_=pst[:])

                    gather_embed(src_idx, NS, emb_src, x_T)
                    gather_embed(tgt_idx, NT, emb_tgt, xt_T)
                    dump_tap(wp, "tap_xT", lambda ch: x_T[:, ch, :], 2, NS)

                    # ---------- gx0 for decoder layer 0 (tokens (t,b)) ----------
                    with tc.tile_pool(name="d0x", bufs=1) as d0p, \
                         tc.tile_pool(name="ps_g0", bufs=2, space="PSUM") as psg0:
                        wx0 = d0p.tile([128, 2, 512], BF, name="wx0")
                        b0s = d0p.tile([128, 4, 1], F32, name="b0s")
                        nc.sync.dma_start(wx0[:], wx_d0[:])
                        nc.sync.dma_start(b0s[:], b_d0[:])
                        gx0f = gx0[:].rearrange("p m t b -> p m (t b)")
                        for ts in range(3):
                            lo = ts * 512
                            for m in range(4):
                                ps = psg0.tile([128, 512], F32, tag="g0ps", name="g0ps")
                                for k in range(2):
                                    nc.tensor.matmul(ps[:], wx0[:, k, m * 128:(m + 1) * 128],
                                                     xt_T[:, k, lo:lo + 512],
                                                     start=(k == 0), stop=(k == 1))
                                nc.vector.tensor_tensor(
                                    out=gx0f[:, m, lo:lo + 512], in0=ps[:],
                                    in1=b0s[:, m, 0:1].to_broadcast([128, 512]), op=ADD)

                    # ---------- phase 1a: encoder layer 1 (needs x_T) ----------
                    for d in "fb":
                        _enc_pass(nc, tc, wp, wx, wh, bb, gx_dram, cfin,
                                  0, d, x_T, x2_T, 2)

                # ---------- phase 1b: encoder layer 2 (embp closed) ----------
                for d in "fb":
                    _enc_pass(nc, tc, wp, wx, wh, bb, gx_dram, cfin,
                              1, d, x2_T, enc_T, 8)

                dump_tap(wp, "tap_x2T", lambda ch: x2_T[:, ch, :], 8, NS)
                dump_tap(wp, "tap_encT", lambda ch: enc_T[:, ch, :], 8, NS)

                # ---------- phase 2: decoder ----------
                with tc.tile_pool(name="decp", bufs=1) as dcp, \
                     tc.tile_pool(name="ps_dec", bufs=2, space="PSUM") as psd:
                    wh0 = dcp.tile([128, 8, 512], BF, name="wh0")
                    w1s = dcp.tile([128, 16, 512], BF, name="w1s")
                    b1s = dcp.tile([128, 4, B], F32, name="b1s")
                    sel = dcp.tile([128, 8, B], F32, name="sel")
                    nc.sync.dma_start(wh0[:], wh_d0[:])
                    nc.sync.dma_start(w1s[:], w_d1[:])
                    nc.sync.dma_start(b1s[:], b_d1[:])
                    nc.sync.dma_start(sel[:], selmask[:])

                    h0T = dcp.tile([128, 8, B], BF, tag="h0Ti", name="h0Ti")
                    h1T = dcp.tile([128, 8, B], BF, tag="h1Ti", name="h1Ti")
                    nc.vector.tensor_copy(out=h0T[:, 0:4, :], in_=x2_T[:, 0:4, (S - 1)::S])
                    nc.vector.tensor_copy(out=h0T[:, 4:8, :], in_=x2_T[:, 4:8, 0::S])
                    nc.vector.tensor_copy(out=h1T[:, 0:4, :], in_=enc_T[:, 0:4, (S - 1)::S])
                    nc.vector.tensor_copy(out=h1T[:, 4:8, :], in_=enc_T[:, 4:8, 0::S])

                    c0full = wp.tile([128, 8, B], F32, tag="cful", name="c0full")
                    c1full = wp.tile([128, 8, B], F32, tag="cful", name="c1full")
                    nc.vector.tensor_copy(out=c0full[:, 0:4, :], in_=cfin[(0, "f")][:])
                    nc.vector.tensor_copy(out=c0full[:, 4:8, :], in_=cfin[(0, "b")][:])
                    nc.vector.tensor_copy(out=c1full[:, 0:4, :], in_=cfin[(1, "f")][:])
                    nc.vector.tensor_copy(out=c1full[:, 4:8, :], in_=cfin[(1, "b")][:])

                    def extract_slice(cfull, out_ap):
                        msk = wp.tile([128, 8, B], F32, tag="mskt", name="mskt")
                        nc.vector.tensor_tensor(out=msk[:], in0=cfull[:], in1=sel[:], op=MUL)
                        nc.vector.tensor_reduce(
                            out=out_ap, in_=msk[:].rearrange("p c b -> p b c"),
                            axis=mybir.AxisListType.X, op=ADD)

                    c0 = dcp.tile([128, 1, B], F32, name="c0")
                    c1 = dcp.tile([128, 1, B], F32, name="c1")
                    extract_slice(c0full, c0[:, 0, :])
                    extract_slice(c1full, c1[:, 0, :])

                    for t in range(T):
                        ps0 = psd.tile([128, 128], F32, tag="dps0", name="dps0")
                        for m in range(4):
                            for k in range(8):
                                nc.tensor.matmul(ps0[:, m * 32:(m + 1) * 32],
                                                 wh0[:, k, m * 128:(m + 1) * 128], h0T[:, k, :],
                                                 start=(k == 0), stop=(k == 7))
                        g0 = wp.tile([128, 4, B], BF, tag="g0", name="g0")
                        nc.vector.tensor_tensor(out=g0[:], in0=ps0[:].rearrange("p (m b) -> p m b", b=B),
                                                in1=gx0[:, :, t, :], op=ADD)
                        h0s = wp.tile([128, 1, B], BF, tag="h0s", name="h0s")
                        _lstm_chain(nc, wp, g0, c0, h0s[:], 1)

                        cin0 = dp.tile([128, B], BF, tag="cin0", name="cin0")
                        cout0 = dp.tile([NC * 128, B], BF, tag="cout0", name="cout0", addr_space="Shared")
                        nc.sync.dma_start(cin0[:], h0s[:, 0, :])
                        nc.gpsimd.collective_compute(
                            "AllGather", mybir.AluOpType.bypass, replica_groups=rg,
                            ins=[cin0[:]], outs=[cout0[:]])
                        h0T = dcp.tile([128, 8, B], BF, tag="h0T", name="h0T", bufs=2)
                        nc.sync.dma_start(h0T[:], cout0.rearrange("(c p) b -> p c b", p=128))

                        ps1 = psd.tile([128, 128], F32, tag="dps1", name="dps1")
                        for m in range(4):
                            for k in range(16):
                                rhs = h0T[:, k, :] if k < 8 else h1T[:, k - 8, :]
                                nc.tensor.matmul(ps1[:, m * 32:(m + 1) * 32],
                                                 w1s[:, k, m * 128:(m + 1) * 128], rhs,
                                                 start=(k == 0), stop=(k == 15))
                        g1 = wp.tile([128, 4, B], BF, tag="g1", name="g1")
                        nc.vector.tensor_tensor(out=g1[:], in0=ps1[:].rearrange("p (m b) -> p m b", b=B),
                                                in1=b1s[:], op=ADD)
                        h1s = wp.tile([128, 1, B], BF, tag="h1s", name="h1s")
                        _lstm_chain(nc, wp, g1, c1, h1s[:], 1)

                        cin1 = dp.tile([128, B], BF, tag="cin1", name="cin1")
                        cout1 = dp.tile([NC * 128, B], BF, tag="cout1", name="cout1", addr_space="Shared")
                        nc.sync.dma_start(cin1[:], h1s[:, 0, :])
                        nc.gpsimd.collective_compute(
                            "AllGather", mybir.AluOpType.bypass, replica_groups=rg,
                            ins=[cin1[:]], outs=[cout1[:]])
                        h1T = dcp.tile([128, 8, B], BF, tag="h1T", name="h1T", bufs=2)
                        nc.sync.dma_start(h1T[:], cout1.rearrange("(c p) b -> p c b", p=128))
                        nc.sync.dma_start(h1_hist[:, :, t, :], cout1.rearrange("(c p) b -> p c b", p=128))

                dump_tap(wp, "tap_h1hist",
                         lambda ch: h1_hist[:, ch, :, :].rearrange("p t b -> p (t b)"), 8, NT)

                # ---------- phase 3: attention ----------
                with tc.tile_pool(name="attp", bufs=1) as ap_, \
                     tc.tile_pool(name="ps_att", bufs=2, space="PSUM") as psa:
                    enc_tok = ap_.tile([128, 16, H2], BF, name="enc_tok")
                    for tck in range(16):
                        pst = psa.tile([128, 8, 128], BF, tag="etr", name="etr")
                        for hc in range(8):
                            nc.tensor.transpose(out=pst[:, hc, :],
                                                in_=enc_T[:, hc, tck * 128:(tck + 1) * 128],
                                                identity=ident[:])
                        nc.vector.tensor_copy(out=enc_tok[:, tck, :],
                                              in_=pst[:].rearrange("p h n -> p (h n)"))

                    for b in range(B):
                        ssc = psa.tile([48, 64], F32, tag="ssc", name="ssc")
                        for k in range(8):
                            nc.tensor.matmul(ssc[:], h1_hist[:, k, :, b], enc_T[:, k, b * S:(b + 1) * S],
                                             start=(k == 0), stop=(k == 7))
                        att_e = wp.tile([48, 64], F32, tag="att_e", name="att_e")
                        nc.scalar.activation(att_e[:], ssc[:], EXP)
                        ssum = wp.tile([48, 1], F32, tag="ssum", name="ssum")
                        nc.vector.tensor_reduce(out=ssum[:], in_=att_e[:],
                                                axis=mybir.AxisListType.X, op=ADD)
                        rs = wp.tile([48, 1], F32, tag="rs", name="rs")
                        nc.vector.reciprocal(rs[:], ssum[:])
                        attn = wp.tile([48, 64], BF, tag="attn", name="attn")
                        nc.vector.tensor_scalar_mul(attn[:], att_e[:], rs[:, 0:1])
                        base = (b % 2) * 64
                        atp = psa.tile([128, 48], BF, tag="atp", name="atp")
                        nc.tensor.transpose(out=atp[base:base + 64, :], in_=attn[:],
                                            identity=ident[0:48, 0:48])
                        attnT = wp.tile([128, 48], BF, tag="attnT", name="attnT")
                        nc.vector.tensor_copy(out=attnT[base:base + 64, :], in_=atp[base:base + 64, :])
                        pctx = psa.tile([128, 8, 48], F32, tag="pctx", name="pctx")
                        tck = b // 2
                        for m in range(8):
                            nc.tensor.matmul(pctx[:, m, :],
                                             enc_tok[base:base + 64, tck, m * 128:(m + 1) * 128],
                                             attnT[base:base + 64, :], start=True, stop=True)
                        nc.vector.tensor_copy(out=ctx_T[:, :, :, b], in_=pctx[:])

            # ---------- phase 4: comb + logits ----------
            dump_tap(wp, "tap_ctxT", lambda ch: ctx_T[:, ch, :, :].rearrange("p t b -> p (t b)"), 8, NT)
            h1h_flat = h1_hist[:].rearrange("p c t b -> p c (t b)")
            ctx_flat = ctx_T[:].rearrange("p c t b -> p c (t b)")
            with tc.tile_pool(name="tailp", bufs=1) as tlp:
                comb_T = tlp.tile([128, 8, NT], BF, name="comb_T")
                with tc.tile_pool(name="combp", bufs=1) as cbp, \
                     tc.tile_pool(name="ps_cmb", bufs=2, space="PSUM") as psc:
                    acw_sb = cbp.tile([128, 16, H2], BF, name="acw_sb")
                    acb_sb = cbp.tile([128, 8, 1], F32, name="acb_sb")
                    nc.sync.dma_start(acw_sb[:], acw_d[:])
                    nc.sync.dma_start(acb_sb[:], acb_d[:])
                    for ts in range(3):
                        lo = ts * 512
                        for m in range(8):
                            ps = psc.tile([128, 512], F32, tag="cps", name="cps")
                            for k in range(16):
                                rhs = h1h_flat[:, k, lo:lo + 512] if k < 8 else ctx_flat[:, k - 8, lo:lo + 512]
                                nc.tensor.matmul(ps[:], acw_sb[:, k, m * 128:(m + 1) * 128], rhs,
                                                 start=(k == 0), stop=(k == 15))
                            nc.vector.tensor_tensor(out=comb_T[:, m, lo:lo + 512], in0=ps[:],
                                                    in1=acb_sb[:, m, 0:1].to_broadcast([128, 512]), op=ADD)

                with tc.tile_pool(name="logp", bufs=1) as lgp, \
                     tc.tile_pool(name="ps_log", bufs=2, space="PSUM") as psl:
                    opw_sb = lgp.tile([128, 8, VP], BF, name="opw_sb")
                    opb_sb = lgp.tile([128, 32, 1], F32, name="opb_sb")
                    nc.sync.dma_start(opw_sb[:], opw_d[:])
                    nc.sync.dma_start(opb_sb[:], opb_d[:])
                    for m in range(32):
                        for ts in range(3):
                            lo = ts * 512
                            ps = psl.tile([128, 512], F32, tag="lps", name="lps")
                            for k in range(8):
                                nc.tensor.matmul(ps[:], opw_sb[:, k, m * 128:(m + 1) * 128],
                                                 comb_T[:, k, lo:lo + 512],
                                                 start=(k == 0), stop=(k == 7))
                            tmp = wp.tile([128, 512], F32, tag="lev", name="lev")
                            nc.vector.tensor_tensor(out=tmp[:], in0=ps[:],
                                                    in1=opb_sb[:, m, 0:1].to_broadcast([128, 512]), op=ADD)
                            nc.sync.dma_start(logits_out[m * 128:(m + 1) * 128, lo:lo + 512], tmp[:])

    split_drain_waits(nc)
    return nc


# ----------------------------------------------------------------------------
# entry point
# ----------------------------------------------------------------------------

_CACHE = {}


def run_spmd(in_maps, debug_taps=()):
    from concourse.bass_utils import run_bass_kernel_spmd
    key = tuple(sorted(n for n, _ in debug_taps))
    if key not in _CACHE:
        _CACHE[key] = build_nc(debug_taps=debug_taps)
    nc = _CACHE[key]
    return run_bass_kernel_spmd(nc, in_maps, core_ids=list(range(NC)))


def kernel(**inputs) -> np.ndarray:
    in_maps = prepare_in_maps(**inputs)
    res = run_spmd(in_maps)
    parts = []
    for c in range(NC):
        lt = res.results[c]["logits_T"][:4000, :]          # [4000, T*B] cols (t,b)
        parts.append(lt.reshape(4000, T, B).transpose(2, 1, 0))
    return np.ascontiguousarray(np.concatenate(parts, axis=2), dtype=np.float32)
